# revision 9
# baseline (speedup 1.0000x reference)
"""Cross-attention layer for Trainium2 (Bass), 8-core data-parallel.

The wall-clock of a call is dominated by host<->device transfers over the
axon tunnel (~30-60 MB/s, partially full-duplex), not device compute
(~0.4 ms/core).  So the kernel is built around minimizing bytes and
round-trips on the wire:

  host (1 cpu, cheap): Q = Wq@Xq+bq, K = Wk@Xk+bk   (0.26% of FLOPs)
                       V -> per-channel int8 (scale amax_c/127)
  H2D per core:        qk fp16 [128,4096] (1 MB), v8 int8 (2 MB)
  device per core:     S = Q^T K (fp16 matmuls, f32 psum)
                       P = softmax(S) (exp w/ accumulated row sums, bf16)
                       outT[q,c] = sum_p P^T[p,q] V8^T[p,c] (bf16, f32 psum)
                       per-query int8 quantization of outT rows; the f32
                       quant multiplier rq is bitcast-packed into the last
                       4 columns so one D2H tensor carries everything
  D2H per core:        o8c int8 [4096, 516] (2.02 MB)
  host:                out[b] = (o8 / rq).T * vscale, pipelined per shard
                       under the D2H stream

Per-query (not per-channel) output scaling matters: attention rows vary
wildly in sharpness, so a channel-wide scale clips diffuse queries. The
device ships back its actual quantization multiplier rq (not a recomputed
reciprocal) so reciprocal-approximation error cancels exactly.

Dispatch is a trimmed run_bass_via_pjrt: one jit(shard_map) over 8 cores
cached at module level (no per-call retrace), with the dead "donated zero
output" operand kept resident on device so no zero bytes cross the tunnel.
"""

import time
from concurrent.futures import ThreadPoolExecutor

import numpy as np

try:
    import concourse.bass as bass  # noqa: F401
except ImportError:  # pragma: no cover - path setup for bare containers
    import sys

    sys.path.insert(0, "/opt/trn_rl_repo")
    import concourse.bass as bass  # noqa: F401

import jax
import jax.numpy as jnp
from jax.experimental.shard_map import shard_map
from jax.sharding import Mesh, NamedSharding, PartitionSpec

import concourse.mybir as mybir
import concourse.tile as tile
from concourse import bacc
from concourse.bass2jax import (
    _bass_exec_p,
    install_neuronx_cc_hook,
    partition_id_tensor,
)
from concourse.masks import make_identity

F32 = mybir.dt.float32
F16 = mybir.dt.float16
BF16 = mybir.dt.bfloat16
I8 = mybir.dt.int8
AF = mybir.ActivationFunctionType
AX = mybir.AxisListType

B = 8
C = 512
HW = 4096
D = 64
N_CORES = 8
OC = C + 4  # o8 columns + packed f32 rq
N_GROUPS = 2  # dispatch split: groups of cores, pipelined for duplex overlap

_TIMINGS = {}


def build_nc(c=C, hw=HW, d=D):
    """Single-core Bass program (SPMD across cores via shard_map)."""
    P = 128
    NKC = c // P          # 128-channel chunks of V
    NSLAB = hw // 512     # 512-wide q-supers
    NPC = hw // P         # 128-wide pixel chunks (transpose granularity)
    QT_PER_QS = 4         # 128-row q-tiles per q-super
    S_W = 1024            # S psum tile width
    N_SH = hw // S_W

    nc = bacc.Bacc("TRN2", target_bir_lowering=False)

    qk_in = nc.dram_tensor("qk", [2 * d, hw], F16, kind="ExternalInput")
    v8_in = nc.dram_tensor("v8", [c, hw], I8, kind="ExternalInput")
    o8_out = nc.dram_tensor("o8c", [hw, c + 4], I8, kind="ExternalOutput")

    with tile.TileContext(nc) as tc:
        with (
            tc.tile_pool(name="const", bufs=1) as const,
            tc.tile_pool(name="persist", bufs=1) as persist,
            tc.tile_pool(name="small", bufs=4) as small,
            tc.tile_pool(name="psT", bufs=2, space="PSUM") as psT,
            tc.tile_pool(name="psV", bufs=2, space="PSUM") as psV,
        ):
            ident = const.tile([P, P], BF16, name="ident")
            make_identity(nc, ident)

            # Q/K in fp16, duplicated to both 64-row halves so S matmuls can
            # alternate PE array halves (overlaps weight load with streaming).
            q_sb = persist.tile([P, hw], F16, name="q_sb")
            nc.sync.dma_start(out=q_sb[0:d, :], in_=qk_in[0:d, :])
            nc.sync.dma_start(out=q_sb[d : 2 * d, :], in_=q_sb[0:d, :])
            k_sb = persist.tile([P, hw], F16, name="k_sb")
            nc.sync.dma_start(out=k_sb[0:d, :], in_=qk_in[d : 2 * d, :])
            nc.sync.dma_start(out=k_sb[d : 2 * d, :], in_=k_sb[0:d, :])

            vt_sb = persist.tile([P, NPC, c], BF16, name="vt_sb")  # V^T

            # ---- phase 1: V load, upcast, transpose ----
            with tc.tile_pool(name="vload", bufs=1) as vload:
                v8t = vload.tile([P, NKC, hw], I8, name="v8t")
                vr = v8_in[:, :].rearrange("(a p) q -> p a q", p=P)
                for kc in range(NKC):
                    nc.sync.dma_start(
                        out=v8t[:, kc : kc + 1, :], in_=vr[:, kc : kc + 1, :]
                    )
                vb = vload.tile([P, NKC, hw], BF16, name="vb")
                for kc in range(NKC):
                    nc.scalar.copy(out=vb[:, kc, :], in_=v8t[:, kc, :])
                for pc in range(NPC):
                    tp = psT.tile([P, c], BF16, name="vt_ps", tag="psT")
                    for kc in range(NKC):
                        nc.tensor.transpose(
                            tp[:, kc * P : (kc + 1) * P],
                            vb[:, kc, pc * P : (pc + 1) * P],
                            ident,
                        )
                    nc.vector.tensor_copy(vt_sb[:, pc, :], tp)

            # ---- phase 2: attention (software-pipelined q-supers) ----
            with (
                tc.tile_pool(name="pp", bufs=2 * QT_PER_QS + 1) as pp,
                tc.tile_pool(name="ptp", bufs=NPC + 2) as ptp,
                tc.tile_pool(name="outp", bufs=4) as outp,
                tc.tile_pool(name="psS", bufs=2, space="PSUM") as psS,
            ):
                def produce(qs):
                    """S + exp + normalize for q-super qs; returns P tiles."""
                    p_tiles = []
                    for qt in range(QT_PER_QS):
                        qg = qs * QT_PER_QS + qt
                        qsl = slice(qg * P, (qg + 1) * P)
                        p_t = pp.tile([P, hw], BF16, name="p_t", tag="p")
                        l8 = small.tile([P, N_SH], F32, name="l8", tag="l8")
                        for sh in range(N_SH):
                            sp = psS.tile([P, S_W], F32, name="s_ps", tag="psS")
                            for j in range(S_W // 512):
                                pb = sh * (S_W // 512) + j
                                h = (pb % 2) * d
                                nc.tensor.matmul(
                                    sp[:, j * 512 : (j + 1) * 512],
                                    q_sb[h : h + d, qsl],
                                    k_sb[h : h + d, pb * 512 : (pb + 1) * 512],
                                    start=True,
                                    stop=True,
                                )
                            nc.scalar.activation(
                                p_t[:, sh * S_W : (sh + 1) * S_W],
                                sp,
                                AF.Exp,
                                accum_out=l8[:, sh : sh + 1],
                            )
                        lsum = small.tile([P, 1], F32, name="lsum", tag="lsum")
                        nc.vector.reduce_sum(lsum, l8, axis=AX.X)
                        rinv = small.tile([P, 1], F32, name="rinv", tag="rinv")
                        nc.vector.reciprocal(rinv, lsum)
                        nc.vector.tensor_scalar_mul(p_t, p_t, rinv)
                        p_tiles.append(p_t)
                    return p_tiles

                def consume(p_tiles, qs):
                    """P^T transposes + outT matmuls + int8 quantize + DMA."""
                    pt_tiles = []
                    for pc in range(NPC):
                        tp = psT.tile([P, 512], BF16, name="pt_ps", tag="psT")
                        for qt in range(QT_PER_QS):
                            nc.tensor.transpose(
                                tp[:, qt * P : (qt + 1) * P],
                                p_tiles[qt][:, pc * P : (pc + 1) * P],
                                ident,
                            )
                        pt_sb = ptp.tile([P, 512], BF16, name="pt_sb", tag="pt")
                        nc.vector.tensor_copy(pt_sb, tp)
                        pt_tiles.append(pt_sb)

                    for qt in range(QT_PER_QS):
                        qg = qs * QT_PER_QS + qt
                        ops = psV.tile([P, c], F32, name="pv_ps", tag="psV")
                        for pc in range(NPC):
                            nc.tensor.matmul(
                                ops,
                                pt_tiles[pc][:, qt * P : (qt + 1) * P],
                                vt_sb[:, pc, :],
                                start=(pc == 0),
                                stop=(pc == NPC - 1),
                            )
                        # per-query int8: rq = 127/absmax(row); o8 = rne(x*rq)
                        am = small.tile([P, 1], F32, name="am", tag="am")
                        nc.vector.tensor_reduce(
                            out=am,
                            in_=ops,
                            op=mybir.AluOpType.max,
                            axis=AX.X,
                            apply_absolute_value=True,
                        )
                        nc.vector.tensor_scalar_max(am, am, 1e-20)
                        rqv = outp.tile([P, 1], F32, name="rqv", tag="rqv")
                        nc.vector.reciprocal(rqv, am)
                        nc.vector.tensor_scalar_mul(rqv, rqv, 127.0)
                        o8t = outp.tile([P, c + 4], I8, name="o8t", tag="o8t")
                        nc.vector.tensor_scalar_mul(o8t[:, 0:c], ops, rqv)
                        nc.vector.tensor_copy(
                            o8t[:, c : c + 4], rqv.bitcast(I8)
                        )
                        nc.sync.dma_start(
                            out=o8_out[qg * P : (qg + 1) * P, :], in_=o8t
                        )

                prev = None
                for qs in range(NSLAB):
                    cur = produce(qs)
                    if prev is not None:
                        consume(*prev)
                    prev = (cur, qs)
                consume(*prev)

    nc.compile()
    return nc


# ---------------------------------------------------------------------------
# dispatch: trimmed run_bass_via_pjrt with cached jit + device-resident zeros
# ---------------------------------------------------------------------------

_STATE = {}


def _cpu():
    return jax.devices("cpu")[0]


def _get_state():
    if "sharded" in _STATE:
        return _STATE

    install_neuronx_cc_hook()
    nc = build_nc()

    partition_name = (
        nc.partition_id_tensor.name if nc.partition_id_tensor else None
    )
    in_names = []
    out_names = []
    out_avals = []
    for alloc in nc.m.functions[0].allocations:
        if not isinstance(alloc, mybir.MemoryLocationSet):
            continue
        name = alloc.memorylocations[0].name
        if alloc.kind == "ExternalInput":
            if name != partition_name:
                in_names.append(name)
        elif alloc.kind == "ExternalOutput":
            out_names.append(name)
            out_avals.append(
                jax.core.ShapedArray(
                    tuple(alloc.tensor_shape), mybir.dt.np(alloc.dtype)
                )
            )
    all_in_names = in_names + out_names
    if partition_name is not None:
        all_in_names.append(partition_name)
    all_in_names = tuple(all_in_names)
    out_avals = tuple(out_avals)
    out_names = tuple(out_names)

    def _body(*args):
        operands = list(args)
        if partition_name is not None:
            operands.append(partition_id_tensor())
        outs = _bass_exec_p.bind(
            *operands,
            out_avals=out_avals,
            in_names=all_in_names,
            out_names=out_names,
            lowering_input_output_aliases=(),
            sim_require_finite=True,
            sim_require_nnan=True,
            nc=nc,
        )
        return tuple(outs)

    devices = jax.devices()[:N_CORES]
    n_args = len(in_names) + len(out_names)
    cpg = N_CORES // N_GROUPS
    groups = []
    for g in range(N_GROUPS):
        mesh = Mesh(np.asarray(devices[g * cpg : (g + 1) * cpg]), ("core",))
        sharded = jax.jit(
            shard_map(
                _body,
                mesh=mesh,
                in_specs=(PartitionSpec("core"),) * n_args,
                out_specs=(PartitionSpec("core"),) * len(out_names),
                check_rep=False,
            ),
            keep_unused=True,
        )
        # Dead "pre-zeroed output" operand the bass_exec convention
        # requires. Kept resident on device; never donated, so reusable.
        zshard = NamedSharding(mesh, PartitionSpec("core"))
        zo8 = jax.jit(
            lambda: jnp.zeros((cpg * HW, OC), jnp.int8), out_shardings=zshard
        )()
        groups.append((sharded, zo8))

    _STATE.update(
        groups=groups, cpg=cpg, in_names=in_names, nc=nc,
        pool=ThreadPoolExecutor(N_CORES),
    )
    return _STATE


def _prep(qf, kf, Wq, bq, Wk, bk):
    Xq = qf.reshape(B, C, HW)
    Xk = kf.reshape(B, C, HW)
    Q = jnp.einsum("bcp,dc->bdp", Xq, Wq) + bq[None, :, None]
    K = jnp.einsum("bcp,dc->bdp", Xk, Wk) + bk[None, :, None]
    qk = jnp.concatenate([Q[:, None], K[:, None]], axis=1)  # (B,2,D,HW)
    qkg = qk.astype(jnp.float16).reshape(B * 2 * D, HW)
    amax = jnp.maximum(
        jnp.max(jnp.abs(Xk), axis=2, keepdims=True), 1e-20
    )  # (B,C,1)
    v8 = (
        jnp.clip(jnp.round(Xk * (127.0 / amax)), -127, 127)
        .astype(jnp.int8)
        .reshape(B * C, HW)
    )
    return qkg, v8, amax / 127.0


_PREP = jax.jit(_prep)


def kernel(query_features, key_features, Wq, bq, Wk, bk, vis_CA=0, **_unused):
    t0 = time.time()
    st = _get_state()
    t1 = time.time()

    qf = np.asarray(query_features, np.float32)
    kf = np.asarray(key_features, np.float32)
    with jax.default_device(_cpu()):
        qkg, v8, vsc = _PREP(
            qf,
            kf,
            np.asarray(Wq, np.float32),
            np.asarray(bq, np.float32),
            np.asarray(Wk, np.float32),
            np.asarray(bk, np.float32),
        )
        qkg, v8 = np.asarray(qkg), np.asarray(v8)
        vscn = np.asarray(vsc)  # (B, C, 1)
    t2 = time.time()

    # Dispatch per core-group (async); D2H of early groups overlaps H2D of
    # later ones (the tunnel is full-duplex), and shard fetch + host post
    # pipeline under the streams.
    cpg = st["cpg"]
    futs = []
    for g, (sharded, zo8) in enumerate(st["groups"]):
        (o8c_g,) = sharded(
            qkg[g * cpg * 2 * D : (g + 1) * cpg * 2 * D],
            v8[g * cpg * C : (g + 1) * cpg * C],
            zo8,
        )
        futs.extend(
            st["pool"].submit(lambda s: np.asarray(s.data), sh)
            for sh in o8c_g.addressable_shards
        )
    out = np.empty((B, C, HW), np.float32)
    t3 = None
    for b, fut in enumerate(futs):
        ob = fut.result()  # (HW, C+4) int8
        if t3 is None:
            t3 = time.time()
        rqb = ob[:, C : C + 4].copy().view(np.float32)  # (HW, 1)
        tmp = ob[:, :C].astype(np.float32) / rqb  # (HW, C)
        out[b] = tmp.T * vscn[b]
    t4 = time.time()

    _TIMINGS.update(
        setup=t1 - t0, prep=t2 - t1, device=(t3 or t4) - t2, fetch_post=t4 - (t3 or t4)
    )
    return out.reshape(B, C, 64, 64)


# revision 10
# speedup vs baseline: 1.5863x; 1.5863x over previous
"""Cross-attention layer for Trainium2 (Bass), 8-core data-parallel.

The wall-clock of a call is dominated by host<->device transfers over the
axon tunnel (~30-60 MB/s, partially full-duplex), not device compute
(~0.4 ms/core).  So the kernel is built around minimizing bytes and
round-trips on the wire:

  host (1 cpu, cheap): Q = Wq@Xq+bq, K = Wk@Xk+bk   (0.26% of FLOPs)
                       V -> per-channel int8 (scale amax_c/127)
  H2D per core:        qk fp16 [128,4096] (1 MB), v8 int8 (2 MB)
  device per core:     S = Q^T K (fp16 matmuls, f32 psum)
                       P = softmax(S) (exp w/ accumulated row sums, bf16)
                       outT[q,c] = sum_p P^T[p,q] V8^T[p,c] (bf16, f32 psum)
                       per-query int8 quantization of outT rows; the f32
                       quant multiplier rq is bitcast-packed into the last
                       4 columns so one D2H tensor carries everything
  D2H per core:        o8c int8 [4096, 516] (2.02 MB)
  host:                out[b] = (o8 / rq).T * vscale, pipelined per shard
                       under the D2H stream

Per-query (not per-channel) output scaling matters: attention rows vary
wildly in sharpness, so a channel-wide scale clips diffuse queries. The
device ships back its actual quantization multiplier rq (not a recomputed
reciprocal) so reciprocal-approximation error cancels exactly.

Dispatch is a trimmed run_bass_via_pjrt: one jit(shard_map) over 8 cores
cached at module level (no per-call retrace), with the dead "donated zero
output" operand kept resident on device so no zero bytes cross the tunnel.
"""

import time
from concurrent.futures import ThreadPoolExecutor

import numpy as np

try:
    import concourse.bass as bass  # noqa: F401
except ImportError:  # pragma: no cover - path setup for bare containers
    import sys

    sys.path.insert(0, "/opt/trn_rl_repo")
    import concourse.bass as bass  # noqa: F401

import jax
import jax.numpy as jnp
from jax.experimental.shard_map import shard_map
from jax.sharding import Mesh, NamedSharding, PartitionSpec

import concourse.mybir as mybir
import concourse.tile as tile
from concourse import bacc
from concourse.bass2jax import (
    _bass_exec_p,
    install_neuronx_cc_hook,
    partition_id_tensor,
)
from concourse.masks import make_identity

F32 = mybir.dt.float32
F16 = mybir.dt.float16
BF16 = mybir.dt.bfloat16
I8 = mybir.dt.int8
AF = mybir.ActivationFunctionType
AX = mybir.AxisListType

B = 8
C = 512
HW = 4096
D = 64
N_CORES = 8
OC = C + 4  # o8 columns + packed f32 rq
N_GROUPS = 2  # dispatch split: groups of cores, pipelined for duplex overlap

_TIMINGS = {}


def build_nc(c=C, hw=HW, d=D):
    """Single-core Bass program (SPMD across cores via shard_map)."""
    P = 128
    NKC = c // P          # 128-channel chunks of V
    NSLAB = hw // 512     # 512-wide q-supers
    NPC = hw // P         # 128-wide pixel chunks (transpose granularity)
    QT_PER_QS = 4         # 128-row q-tiles per q-super
    S_W = 1024            # S psum tile width
    N_SH = hw // S_W

    nc = bacc.Bacc("TRN2", target_bir_lowering=False)

    qk_in = nc.dram_tensor("qk", [2 * d, hw], F16, kind="ExternalInput")
    v8_in = nc.dram_tensor("v8", [c, hw], I8, kind="ExternalInput")
    o8_out = nc.dram_tensor("o8c", [hw, c + 4], I8, kind="ExternalOutput")

    with tile.TileContext(nc) as tc:
        with (
            tc.tile_pool(name="const", bufs=1) as const,
            tc.tile_pool(name="persist", bufs=1) as persist,
            tc.tile_pool(name="small", bufs=4) as small,
            tc.tile_pool(name="psT", bufs=2, space="PSUM") as psT,
            tc.tile_pool(name="psV", bufs=2, space="PSUM") as psV,
        ):
            ident = const.tile([P, P], BF16, name="ident")
            make_identity(nc, ident)

            # Q/K in fp16, duplicated to both 64-row halves so S matmuls can
            # alternate PE array halves (overlaps weight load with streaming).
            q_sb = persist.tile([P, hw], F16, name="q_sb")
            nc.sync.dma_start(out=q_sb[0:d, :], in_=qk_in[0:d, :])
            nc.sync.dma_start(out=q_sb[d : 2 * d, :], in_=q_sb[0:d, :])
            k_sb = persist.tile([P, hw], F16, name="k_sb")
            nc.sync.dma_start(out=k_sb[0:d, :], in_=qk_in[d : 2 * d, :])
            nc.sync.dma_start(out=k_sb[d : 2 * d, :], in_=k_sb[0:d, :])

            vt_sb = persist.tile([P, NPC, c], BF16, name="vt_sb")  # V^T

            # ---- phase 1: V load, upcast, transpose ----
            with tc.tile_pool(name="vload", bufs=1) as vload:
                v8t = vload.tile([P, NKC, hw], I8, name="v8t")
                vr = v8_in[:, :].rearrange("(a p) q -> p a q", p=P)
                for kc in range(NKC):
                    nc.sync.dma_start(
                        out=v8t[:, kc : kc + 1, :], in_=vr[:, kc : kc + 1, :]
                    )
                vb = vload.tile([P, NKC, hw], BF16, name="vb")
                for kc in range(NKC):
                    nc.scalar.copy(out=vb[:, kc, :], in_=v8t[:, kc, :])
                for pc in range(NPC):
                    tp = psT.tile([P, c], BF16, name="vt_ps", tag="psT")
                    for kc in range(NKC):
                        nc.tensor.transpose(
                            tp[:, kc * P : (kc + 1) * P],
                            vb[:, kc, pc * P : (pc + 1) * P],
                            ident,
                        )
                    nc.vector.tensor_copy(vt_sb[:, pc, :], tp)

            # ---- phase 2: attention (software-pipelined q-supers) ----
            with (
                tc.tile_pool(name="pp", bufs=2 * QT_PER_QS + 1) as pp,
                tc.tile_pool(name="ptp", bufs=NPC + 2) as ptp,
                tc.tile_pool(name="outp", bufs=4) as outp,
                tc.tile_pool(name="psS", bufs=2, space="PSUM") as psS,
            ):
                def produce(qs):
                    """S + exp + normalize for q-super qs; returns P tiles."""
                    p_tiles = []
                    for qt in range(QT_PER_QS):
                        qg = qs * QT_PER_QS + qt
                        qsl = slice(qg * P, (qg + 1) * P)
                        p_t = pp.tile([P, hw], BF16, name="p_t", tag="p")
                        l8 = small.tile([P, N_SH], F32, name="l8", tag="l8")
                        for sh in range(N_SH):
                            sp = psS.tile([P, S_W], F32, name="s_ps", tag="psS")
                            for j in range(S_W // 512):
                                pb = sh * (S_W // 512) + j
                                h = (pb % 2) * d
                                nc.tensor.matmul(
                                    sp[:, j * 512 : (j + 1) * 512],
                                    q_sb[h : h + d, qsl],
                                    k_sb[h : h + d, pb * 512 : (pb + 1) * 512],
                                    start=True,
                                    stop=True,
                                )
                            nc.scalar.activation(
                                p_t[:, sh * S_W : (sh + 1) * S_W],
                                sp,
                                AF.Exp,
                                accum_out=l8[:, sh : sh + 1],
                            )
                        lsum = small.tile([P, 1], F32, name="lsum", tag="lsum")
                        nc.vector.reduce_sum(lsum, l8, axis=AX.X)
                        rinv = small.tile([P, 1], F32, name="rinv", tag="rinv")
                        nc.vector.reciprocal(rinv, lsum)
                        nc.vector.tensor_scalar_mul(p_t, p_t, rinv)
                        p_tiles.append(p_t)
                    return p_tiles

                def consume(p_tiles, qs):
                    """P^T transposes + outT matmuls + int8 quantize + DMA."""
                    pt_tiles = []
                    for pc in range(NPC):
                        tp = psT.tile([P, 512], BF16, name="pt_ps", tag="psT")
                        for qt in range(QT_PER_QS):
                            nc.tensor.transpose(
                                tp[:, qt * P : (qt + 1) * P],
                                p_tiles[qt][:, pc * P : (pc + 1) * P],
                                ident,
                            )
                        pt_sb = ptp.tile([P, 512], BF16, name="pt_sb", tag="pt")
                        nc.vector.tensor_copy(pt_sb, tp)
                        pt_tiles.append(pt_sb)

                    for qt in range(QT_PER_QS):
                        qg = qs * QT_PER_QS + qt
                        ops = psV.tile([P, c], F32, name="pv_ps", tag="psV")
                        for pc in range(NPC):
                            nc.tensor.matmul(
                                ops,
                                pt_tiles[pc][:, qt * P : (qt + 1) * P],
                                vt_sb[:, pc, :],
                                start=(pc == 0),
                                stop=(pc == NPC - 1),
                            )
                        # per-query int8: rq = 127/absmax(row); o8 = rne(x*rq)
                        am = small.tile([P, 1], F32, name="am", tag="am")
                        nc.vector.tensor_reduce(
                            out=am,
                            in_=ops,
                            op=mybir.AluOpType.max,
                            axis=AX.X,
                            apply_absolute_value=True,
                        )
                        nc.vector.tensor_scalar_max(am, am, 1e-20)
                        rqv = outp.tile([P, 1], F32, name="rqv", tag="rqv")
                        nc.vector.reciprocal(rqv, am)
                        nc.vector.tensor_scalar_mul(rqv, rqv, 127.0)
                        o8t = outp.tile([P, c + 4], I8, name="o8t", tag="o8t")
                        nc.vector.tensor_scalar_mul(o8t[:, 0:c], ops, rqv)
                        nc.vector.tensor_copy(
                            o8t[:, c : c + 4], rqv.bitcast(I8)
                        )
                        nc.sync.dma_start(
                            out=o8_out[qg * P : (qg + 1) * P, :], in_=o8t
                        )

                prev = None
                for qs in range(NSLAB):
                    cur = produce(qs)
                    if prev is not None:
                        consume(*prev)
                    prev = (cur, qs)
                consume(*prev)

    nc.compile()
    return nc


# ---------------------------------------------------------------------------
# dispatch: trimmed run_bass_via_pjrt with cached jit + device-resident zeros
# ---------------------------------------------------------------------------

_STATE = {}


def _cpu():
    return jax.devices("cpu")[0]


def _get_state():
    if "groups" in _STATE:
        return _STATE

    install_neuronx_cc_hook()
    nc = build_nc()

    partition_name = (
        nc.partition_id_tensor.name if nc.partition_id_tensor else None
    )
    in_names = []
    out_names = []
    out_avals = []
    for alloc in nc.m.functions[0].allocations:
        if not isinstance(alloc, mybir.MemoryLocationSet):
            continue
        name = alloc.memorylocations[0].name
        if alloc.kind == "ExternalInput":
            if name != partition_name:
                in_names.append(name)
        elif alloc.kind == "ExternalOutput":
            out_names.append(name)
            out_avals.append(
                jax.core.ShapedArray(
                    tuple(alloc.tensor_shape), mybir.dt.np(alloc.dtype)
                )
            )
    all_in_names = in_names + out_names
    if partition_name is not None:
        all_in_names.append(partition_name)
    all_in_names = tuple(all_in_names)
    out_avals = tuple(out_avals)
    out_names = tuple(out_names)

    def _body(*args):
        operands = list(args)
        if partition_name is not None:
            operands.append(partition_id_tensor())
        outs = _bass_exec_p.bind(
            *operands,
            out_avals=out_avals,
            in_names=all_in_names,
            out_names=out_names,
            lowering_input_output_aliases=(),
            sim_require_finite=True,
            sim_require_nnan=True,
            nc=nc,
        )
        return tuple(outs)

    devices = jax.devices()[:N_CORES]
    n_args = len(in_names) + len(out_names)
    cpg = N_CORES // N_GROUPS
    groups = []
    for g in range(N_GROUPS):
        mesh = Mesh(np.asarray(devices[g * cpg : (g + 1) * cpg]), ("core",))
        sharded = jax.jit(
            shard_map(
                _body,
                mesh=mesh,
                in_specs=(PartitionSpec("core"),) * n_args,
                out_specs=(PartitionSpec("core"),) * len(out_names),
                check_rep=False,
            ),
            keep_unused=True,
        )
        # Dead "pre-zeroed output" operand the bass_exec convention
        # requires. Kept resident on device; never donated, so reusable.
        zshard = NamedSharding(mesh, PartitionSpec("core"))
        zo8 = jax.jit(
            lambda: jnp.zeros((cpg * HW, OC), jnp.int8), out_shardings=zshard
        )()
        groups.append((sharded, zo8))

    _STATE.update(
        groups=groups, cpg=cpg, in_names=in_names, nc=nc,
        pool=ThreadPoolExecutor(N_CORES),
    )
    return _STATE


def _prep(qf, kf, Wq, bq, Wk, bk):
    Xq = qf.reshape(B, C, HW)
    Xk = kf.reshape(B, C, HW)
    Q = jnp.einsum("bcp,dc->bdp", Xq, Wq) + bq[None, :, None]
    K = jnp.einsum("bcp,dc->bdp", Xk, Wk) + bk[None, :, None]
    qk = jnp.concatenate([Q[:, None], K[:, None]], axis=1)  # (B,2,D,HW)
    qkg = qk.astype(jnp.float16).reshape(B * 2 * D, HW)
    amax = jnp.maximum(
        jnp.max(jnp.abs(Xk), axis=2, keepdims=True), 1e-20
    )  # (B,C,1)
    v8 = (
        jnp.clip(jnp.round(Xk * (127.0 / amax)), -127, 127)
        .astype(jnp.int8)
        .reshape(B * C, HW)
    )
    return qkg, v8, amax / 127.0


_PREP = jax.jit(_prep)


def kernel(query_features, key_features, Wq, bq, Wk, bk, vis_CA=0, **_unused):
    t0 = time.time()
    st = _get_state()
    t1 = time.time()

    qf = np.asarray(query_features, np.float32)
    kf = np.asarray(key_features, np.float32)
    with jax.default_device(_cpu()):
        qkg, v8, vsc = _PREP(
            qf,
            kf,
            np.asarray(Wq, np.float32),
            np.asarray(bq, np.float32),
            np.asarray(Wk, np.float32),
            np.asarray(bk, np.float32),
        )
        qkg, v8 = np.asarray(qkg), np.asarray(v8)
        vscn = np.asarray(vsc)  # (B, C, 1)
    t2 = time.time()

    # Dispatch per core-group (async); D2H of early groups overlaps H2D of
    # later ones (the tunnel is full-duplex), and shard fetch + host post
    # pipeline under the streams.
    cpg = st["cpg"]
    futs = []
    for g, (sharded, zo8) in enumerate(st["groups"]):
        (o8c_g,) = sharded(
            qkg[g * cpg * 2 * D : (g + 1) * cpg * 2 * D],
            v8[g * cpg * C : (g + 1) * cpg * C],
            zo8,
        )
        futs.extend(
            st["pool"].submit(lambda s: np.asarray(s.data), sh)
            for sh in o8c_g.addressable_shards
        )
    out = np.empty((B, C, HW), np.float32)
    t3 = None
    for b, fut in enumerate(futs):
        ob = fut.result()  # (HW, C+4) int8
        if t3 is None:
            t3 = time.time()
        rqb = ob[:, C : C + 4].copy().view(np.float32)  # (HW, 1)
        tmp = ob[:, :C].astype(np.float32) / rqb  # (HW, C)
        out[b] = tmp.T * vscn[b]
    t4 = time.time()

    _TIMINGS.update(
        setup=t1 - t0, prep=t2 - t1, device=(t3 or t4) - t2, fetch_post=t4 - (t3 or t4)
    )
    return out.reshape(B, C, 64, 64)


# revision 15
# speedup vs baseline: 1.6336x; 1.0298x over previous
"""Cross-attention layer for Trainium2 (Bass), 8-core data-parallel.

The wall-clock of a call is dominated by host<->device transfers over the
axon tunnel (~30-60 MB/s, partially full-duplex), not device compute
(~0.4 ms/core).  So the kernel is built around minimizing bytes and
round-trips on the wire:

  host (1 cpu, cheap): Q = Wq@Xq+bq, K = Wk@Xk+bk   (0.26% of FLOPs)
                       V -> per-channel int8 (scale amax_c/127)
  H2D per core:        qk fp16 [128,4096] (1 MB), v8 int8 (2 MB)
  device per core:     S = Q^T K (fp16 matmuls, f32 psum)
                       P = softmax(S) (exp w/ accumulated row sums, bf16)
                       outT[q,c] = sum_p P^T[p,q] V8^T[p,c] (bf16, f32 psum)
                       per-query int8 quantization of outT rows; the f32
                       quant multiplier rq is bitcast-packed into the last
                       4 columns so one D2H tensor carries everything
  D2H per core:        o8c int8 [4096, 516] (2.02 MB)
  host:                out[b] = (o8 / rq).T * vscale, pipelined per shard
                       under the D2H stream

Per-query (not per-channel) output scaling matters: attention rows vary
wildly in sharpness, so a channel-wide scale clips diffuse queries. The
device ships back its actual quantization multiplier rq (not a recomputed
reciprocal) so reciprocal-approximation error cancels exactly.

Dispatch is a trimmed run_bass_via_pjrt: one jit(shard_map) over 8 cores
cached at module level (no per-call retrace), with the dead "donated zero
output" operand kept resident on device so no zero bytes cross the tunnel.
"""

import time
from concurrent.futures import ThreadPoolExecutor

import numpy as np

try:
    import concourse.bass as bass  # noqa: F401
except ImportError:  # pragma: no cover - path setup for bare containers
    import sys

    sys.path.insert(0, "/opt/trn_rl_repo")
    import concourse.bass as bass  # noqa: F401

import jax
import jax.numpy as jnp
from jax.experimental.shard_map import shard_map
from jax.sharding import Mesh, NamedSharding, PartitionSpec

import concourse.mybir as mybir
import concourse.tile as tile
from concourse import bacc
from concourse.bass2jax import (
    _bass_exec_p,
    install_neuronx_cc_hook,
    partition_id_tensor,
)
from concourse.masks import make_identity

F32 = mybir.dt.float32
F16 = mybir.dt.float16
BF16 = mybir.dt.bfloat16
I8 = mybir.dt.int8
AF = mybir.ActivationFunctionType
AX = mybir.AxisListType

B = 8
C = 512
HW = 4096
D = 64
N_CORES = 8
OC = C + 4  # o8 columns + packed f32 rq
N_GROUPS = 4  # dispatch split: groups of cores, pipelined for duplex overlap

_TIMINGS = {}


def build_nc(c=C, hw=HW, d=D):
    """Single-core Bass program (SPMD across cores via shard_map)."""
    P = 128
    NKC = c // P          # 128-channel chunks of V
    NSLAB = hw // 512     # 512-wide q-supers
    NPC = hw // P         # 128-wide pixel chunks (transpose granularity)
    QT_PER_QS = 4         # 128-row q-tiles per q-super
    S_W = 1024            # S psum tile width
    N_SH = hw // S_W

    nc = bacc.Bacc("TRN2", target_bir_lowering=False)

    qk_in = nc.dram_tensor("qk", [2 * d, hw], F16, kind="ExternalInput")
    v8_in = nc.dram_tensor("v8", [c, hw], I8, kind="ExternalInput")
    vsc_in = nc.dram_tensor("vsc", [c, 1], F32, kind="ExternalInput")
    o8_out = nc.dram_tensor("o8c", [hw, c + 4], I8, kind="ExternalOutput")

    with tile.TileContext(nc) as tc:
        with (
            tc.tile_pool(name="const", bufs=1) as const,
            tc.tile_pool(name="persist", bufs=1) as persist,
            tc.tile_pool(name="small", bufs=4) as small,
            tc.tile_pool(name="psT", bufs=2, space="PSUM") as psT,
            tc.tile_pool(name="psV", bufs=2, space="PSUM") as psV,
        ):
            ident = const.tile([P, P], BF16, name="ident")
            make_identity(nc, ident)

            # Q/K in fp16, duplicated to both 64-row halves so S matmuls can
            # alternate PE array halves (overlaps weight load with streaming).
            q_sb = persist.tile([P, hw], F16, name="q_sb")
            nc.sync.dma_start(out=q_sb[0:d, :], in_=qk_in[0:d, :])
            nc.sync.dma_start(out=q_sb[d : 2 * d, :], in_=q_sb[0:d, :])
            k_sb = persist.tile([P, hw], F16, name="k_sb")
            nc.sync.dma_start(out=k_sb[0:d, :], in_=qk_in[d : 2 * d, :])
            nc.sync.dma_start(out=k_sb[d : 2 * d, :], in_=k_sb[0:d, :])

            vt_sb = persist.tile([P, NPC, c], BF16, name="vt_sb")  # V^T

            # ---- phase 1: V load, upcast (channel-scaled), transpose ----
            with tc.tile_pool(name="vload", bufs=1) as vload:
                vsc_sb = vload.tile([P, NKC, 1], F32, name="vsc_sb")
                nc.sync.dma_start(
                    out=vsc_sb,
                    in_=vsc_in[:, :].rearrange("(a p) q -> p a q", p=P),
                )
                v8t = vload.tile([P, NKC, hw], I8, name="v8t")
                vr = v8_in[:, :].rearrange("(a p) q -> p a q", p=P)
                for kc in range(NKC):
                    nc.sync.dma_start(
                        out=v8t[:, kc : kc + 1, :], in_=vr[:, kc : kc + 1, :]
                    )
                vb = vload.tile([P, NKC, hw], BF16, name="vb")
                for kc in range(NKC):
                    nc.scalar.activation(
                        vb[:, kc, :],
                        v8t[:, kc, :],
                        AF.Copy,
                        scale=vsc_sb[:, kc, :],
                    )
                for pc in range(NPC):
                    tp = psT.tile([P, c], BF16, name="vt_ps", tag="psT")
                    for kc in range(NKC):
                        nc.tensor.transpose(
                            tp[:, kc * P : (kc + 1) * P],
                            vb[:, kc, pc * P : (pc + 1) * P],
                            ident,
                        )
                    nc.vector.tensor_copy(vt_sb[:, pc, :], tp)

            # ---- phase 2: attention (software-pipelined q-supers) ----
            with (
                tc.tile_pool(name="pp", bufs=2 * QT_PER_QS + 1) as pp,
                tc.tile_pool(name="ptp", bufs=NPC + 2) as ptp,
                tc.tile_pool(name="outp", bufs=4) as outp,
                tc.tile_pool(name="psS", bufs=2, space="PSUM") as psS,
            ):
                def produce(qs):
                    """S + exp + normalize for q-super qs; returns P tiles."""
                    p_tiles = []
                    for qt in range(QT_PER_QS):
                        qg = qs * QT_PER_QS + qt
                        qsl = slice(qg * P, (qg + 1) * P)
                        p_t = pp.tile([P, hw], BF16, name="p_t", tag="p")
                        l8 = small.tile([P, N_SH], F32, name="l8", tag="l8")
                        for sh in range(N_SH):
                            sp = psS.tile([P, S_W], F32, name="s_ps", tag="psS")
                            for j in range(S_W // 512):
                                pb = sh * (S_W // 512) + j
                                h = (pb % 2) * d
                                nc.tensor.matmul(
                                    sp[:, j * 512 : (j + 1) * 512],
                                    q_sb[h : h + d, qsl],
                                    k_sb[h : h + d, pb * 512 : (pb + 1) * 512],
                                    start=True,
                                    stop=True,
                                )
                            nc.scalar.activation(
                                p_t[:, sh * S_W : (sh + 1) * S_W],
                                sp,
                                AF.Exp,
                                accum_out=l8[:, sh : sh + 1],
                            )
                        lsum = small.tile([P, 1], F32, name="lsum", tag="lsum")
                        nc.vector.reduce_sum(lsum, l8, axis=AX.X)
                        rinv = small.tile([P, 1], F32, name="rinv", tag="rinv")
                        nc.vector.reciprocal(rinv, lsum)
                        nc.vector.tensor_scalar_mul(p_t, p_t, rinv)
                        p_tiles.append(p_t)
                    return p_tiles

                def consume(p_tiles, qs):
                    """P^T transposes + outT matmuls + int8 quantize + DMA."""
                    pt_tiles = []
                    for pc in range(NPC):
                        tp = psT.tile([P, 512], BF16, name="pt_ps", tag="psT")
                        for qt in range(QT_PER_QS):
                            nc.tensor.transpose(
                                tp[:, qt * P : (qt + 1) * P],
                                p_tiles[qt][:, pc * P : (pc + 1) * P],
                                ident,
                            )
                        pt_sb = ptp.tile([P, 512], BF16, name="pt_sb", tag="pt")
                        nc.vector.tensor_copy(pt_sb, tp)
                        pt_tiles.append(pt_sb)

                    for qt in range(QT_PER_QS):
                        qg = qs * QT_PER_QS + qt
                        ops = psV.tile([P, c], F32, name="pv_ps", tag="psV")
                        for pc in range(NPC):
                            nc.tensor.matmul(
                                ops,
                                pt_tiles[pc][:, qt * P : (qt + 1) * P],
                                vt_sb[:, pc, :],
                                start=(pc == 0),
                                stop=(pc == NPC - 1),
                            )
                        # per-query int8: rq = 127/absmax(row); o8 = rne(x*rq)
                        am = small.tile([P, 1], F32, name="am", tag="am")
                        nc.vector.tensor_reduce(
                            out=am,
                            in_=ops,
                            op=mybir.AluOpType.max,
                            axis=AX.X,
                            apply_absolute_value=True,
                        )
                        nc.vector.tensor_scalar_max(am, am, 1e-20)
                        rqv = outp.tile([P, 1], F32, name="rqv", tag="rqv")
                        nc.vector.reciprocal(rqv, am)
                        nc.vector.tensor_scalar_mul(rqv, rqv, 127.0)
                        o8t = outp.tile([P, c + 4], I8, name="o8t", tag="o8t")
                        nc.vector.tensor_scalar_mul(o8t[:, 0:c], ops, rqv)
                        nc.vector.tensor_copy(
                            o8t[:, c : c + 4], rqv.bitcast(I8)
                        )
                        nc.sync.dma_start(
                            out=o8_out[qg * P : (qg + 1) * P, :], in_=o8t
                        )

                prev = None
                for qs in range(NSLAB):
                    cur = produce(qs)
                    if prev is not None:
                        consume(*prev)
                    prev = (cur, qs)
                consume(*prev)

    nc.compile()
    return nc


# ---------------------------------------------------------------------------
# dispatch: trimmed run_bass_via_pjrt with cached jit + device-resident zeros
# ---------------------------------------------------------------------------

_STATE = {}


def _cpu():
    return jax.devices("cpu")[0]


def _get_state():
    if "groups" in _STATE:
        return _STATE

    install_neuronx_cc_hook()
    nc = build_nc()

    partition_name = (
        nc.partition_id_tensor.name if nc.partition_id_tensor else None
    )
    in_names = []
    out_names = []
    out_avals = []
    for alloc in nc.m.functions[0].allocations:
        if not isinstance(alloc, mybir.MemoryLocationSet):
            continue
        name = alloc.memorylocations[0].name
        if alloc.kind == "ExternalInput":
            if name != partition_name:
                in_names.append(name)
        elif alloc.kind == "ExternalOutput":
            out_names.append(name)
            out_avals.append(
                jax.core.ShapedArray(
                    tuple(alloc.tensor_shape), mybir.dt.np(alloc.dtype)
                )
            )
    all_in_names = in_names + out_names
    if partition_name is not None:
        all_in_names.append(partition_name)
    all_in_names = tuple(all_in_names)
    out_avals = tuple(out_avals)
    out_names = tuple(out_names)

    def _body(*args):
        operands = list(args)
        if partition_name is not None:
            operands.append(partition_id_tensor())
        outs = _bass_exec_p.bind(
            *operands,
            out_avals=out_avals,
            in_names=all_in_names,
            out_names=out_names,
            lowering_input_output_aliases=(),
            sim_require_finite=True,
            sim_require_nnan=True,
            nc=nc,
        )
        return tuple(outs)

    devices = jax.devices()[:N_CORES]
    n_args = len(in_names) + len(out_names)
    cpg = N_CORES // N_GROUPS
    groups = []
    for g in range(N_GROUPS):
        mesh = Mesh(np.asarray(devices[g * cpg : (g + 1) * cpg]), ("core",))
        sharded = jax.jit(
            shard_map(
                _body,
                mesh=mesh,
                in_specs=(PartitionSpec("core"),) * n_args,
                out_specs=(PartitionSpec("core"),) * len(out_names),
                check_rep=False,
            ),
            keep_unused=True,
        )
        # Dead "pre-zeroed output" operand the bass_exec convention
        # requires. Kept resident on device; never donated, so reusable.
        zshard = NamedSharding(mesh, PartitionSpec("core"))
        zo8 = jax.jit(
            lambda: jnp.zeros((cpg * HW, OC), jnp.int8), out_shardings=zshard
        )()
        groups.append((sharded, zo8))

    _STATE.update(
        groups=groups, cpg=cpg, in_names=in_names, nc=nc,
        pool=ThreadPoolExecutor(N_CORES),
    )
    return _STATE


def _prep(qf, kf, Wq, bq, Wk):
    # bk is dropped: softmax over keys is invariant to the per-query
    # constant q_i . bk that the K bias adds to every logit in a row.
    Xq = qf.reshape(B, C, HW)
    Xk = kf.reshape(B, C, HW)
    Q = jnp.einsum("bcp,dc->bdp", Xq, Wq) + bq[None, :, None]
    K = jnp.einsum("bcp,dc->bdp", Xk, Wk)
    qk = jnp.concatenate([Q[:, None], K[:, None]], axis=1)  # (B,2,D,HW)
    qkg = qk.astype(jnp.float16).reshape(B * 2 * D, HW)
    amax = jnp.maximum(
        jnp.max(jnp.abs(Xk), axis=2, keepdims=True), 1e-20
    )  # (B,C,1)
    v8 = (
        jnp.clip(jnp.round(Xk * (127.0 / amax)), -127, 127)
        .astype(jnp.int8)
        .reshape(B * C, HW)
    )
    return qkg, v8, (amax / 127.0).reshape(B * C, 1)


_PREP = jax.jit(_prep)


def kernel(query_features, key_features, Wq, bq, Wk, bk, vis_CA=0, **_unused):
    t0 = time.time()
    st = _get_state()
    t1 = time.time()

    qf = np.asarray(query_features, np.float32)
    kf = np.asarray(key_features, np.float32)
    with jax.default_device(_cpu()):
        qkg, v8, vsc = _PREP(
            qf,
            kf,
            np.asarray(Wq, np.float32),
            np.asarray(bq, np.float32),
            np.asarray(Wk, np.float32),
        )
        qkg, v8 = np.asarray(qkg), np.asarray(v8)
        vscn = np.asarray(vsc)  # (B*C, 1)
    t2 = time.time()

    # Dispatch per core-group (async); D2H of early groups overlaps H2D of
    # later ones (the tunnel is full-duplex), and shard fetch + host post
    # pipeline under the streams.
    cpg = st["cpg"]
    futs = []
    for g, (sharded, zo8) in enumerate(st["groups"]):
        (o8c_g,) = sharded(
            qkg[g * cpg * 2 * D : (g + 1) * cpg * 2 * D],
            v8[g * cpg * C : (g + 1) * cpg * C],
            vscn[g * cpg * C : (g + 1) * cpg * C],
            zo8,
        )
        futs.extend(
            st["pool"].submit(lambda s: np.asarray(s.data), sh)
            for sh in o8c_g.addressable_shards
        )
    out = np.empty((B, C, HW), np.float32)
    t3 = None
    for b, fut in enumerate(futs):
        ob = fut.result()  # (HW, C+4) int8
        if t3 is None:
            t3 = time.time()
        rinv = 1.0 / ob[:, C : C + 4].copy().view(np.float32)  # (HW, 1)
        tmp = ob[:, :C].T.astype(np.float32)  # (C, HW)
        np.multiply(tmp, rinv.reshape(1, HW), out=out[b])
    t4 = time.time()

    _TIMINGS.update(
        setup=t1 - t0, prep=t2 - t1, device=(t3 or t4) - t2, fetch_post=t4 - (t3 or t4)
    )
    return out.reshape(B, C, 64, 64)


# revision 18
# speedup vs baseline: 1.8096x; 1.1077x over previous
"""Cross-attention layer for Trainium2 (Bass), 8-core data-parallel.

The wall-clock of a call is dominated by host<->device transfers over the
axon tunnel (~30-60 MB/s, partially full-duplex), not device compute
(~0.4 ms/core).  So the kernel is built around minimizing bytes and
round-trips on the wire:

  host (1 cpu, cheap): Q = Wq@Xq+bq, K = Wk@Xk+bk   (0.26% of FLOPs)
                       V -> per-channel int8 (scale amax_c/127)
  H2D per core:        qk fp16 [128,4096] (1 MB), v8 int8 (2 MB)
  device per core:     S = Q^T K (fp16 matmuls, f32 psum)
                       P = softmax(S) (exp w/ accumulated row sums, bf16)
                       outT[q,c] = sum_p P^T[p,q] V8^T[p,c] (bf16, f32 psum)
                       per-query int8 quantization of outT rows; the f32
                       quant multiplier rq is bitcast-packed into the last
                       4 columns so one D2H tensor carries everything
  D2H per core:        o8c int8 [4096, 516] (2.02 MB)
  host:                out[b] = (o8 / rq).T * vscale, pipelined per shard
                       under the D2H stream

Per-query (not per-channel) output scaling matters: attention rows vary
wildly in sharpness, so a channel-wide scale clips diffuse queries. The
device ships back its actual quantization multiplier rq (not a recomputed
reciprocal) so reciprocal-approximation error cancels exactly.

Dispatch is a trimmed run_bass_via_pjrt: one jit(shard_map) over 8 cores
cached at module level (no per-call retrace), with the dead "donated zero
output" operand kept resident on device so no zero bytes cross the tunnel.
"""

import time
from concurrent.futures import ThreadPoolExecutor

import numpy as np

try:
    import concourse.bass as bass  # noqa: F401
except ImportError:  # pragma: no cover - path setup for bare containers
    import sys

    sys.path.insert(0, "/opt/trn_rl_repo")
    import concourse.bass as bass  # noqa: F401

import jax
import jax.numpy as jnp
from jax.experimental.shard_map import shard_map
from jax.sharding import Mesh, NamedSharding, PartitionSpec

import concourse.mybir as mybir
import concourse.tile as tile
from concourse import bacc
from concourse.bass2jax import (
    _bass_exec_p,
    install_neuronx_cc_hook,
    partition_id_tensor,
)
from concourse.masks import make_identity

F32 = mybir.dt.float32
F16 = mybir.dt.float16
BF16 = mybir.dt.bfloat16
I8 = mybir.dt.int8
AF = mybir.ActivationFunctionType
AX = mybir.AxisListType

B = 8
C = 512
HW = 4096
D = 64
N_CORES = 8
OC = C + 4  # o8 columns + packed f32 rq
N_GROUPS = 4  # dispatch split: groups of cores, pipelined for duplex overlap

_TIMINGS = {}


def build_nc(c=C, hw=HW, d=D):
    """Single-core Bass program (SPMD across cores via shard_map)."""
    P = 128
    NKC = c // P          # 128-channel chunks of V
    NSLAB = hw // 512     # 512-wide q-supers
    NPC = hw // P         # 128-wide pixel chunks (transpose granularity)
    QT_PER_QS = 4         # 128-row q-tiles per q-super
    S_W = 1024            # S psum tile width
    N_SH = hw // S_W

    nc = bacc.Bacc("TRN2", target_bir_lowering=False)

    qk_in = nc.dram_tensor("qk", [2 * d, hw], F16, kind="ExternalInput")
    v8_in = nc.dram_tensor("v8", [c, hw], I8, kind="ExternalInput")
    vsc_in = nc.dram_tensor("vsc", [c, 1], F32, kind="ExternalInput")
    o8_out = nc.dram_tensor("o8c", [hw, c + 4], I8, kind="ExternalOutput")

    with tile.TileContext(nc) as tc:
        with (
            tc.tile_pool(name="const", bufs=1) as const,
            tc.tile_pool(name="persist", bufs=1) as persist,
            tc.tile_pool(name="small", bufs=4) as small,
            tc.tile_pool(name="psT", bufs=2, space="PSUM") as psT,
            tc.tile_pool(name="psV", bufs=2, space="PSUM") as psV,
        ):
            ident = const.tile([P, P], BF16, name="ident")
            make_identity(nc, ident)

            # Q/K in fp16, duplicated to both 64-row halves so S matmuls can
            # alternate PE array halves (overlaps weight load with streaming).
            q_sb = persist.tile([P, hw], F16, name="q_sb")
            nc.sync.dma_start(out=q_sb[0:d, :], in_=qk_in[0:d, :])
            nc.sync.dma_start(out=q_sb[d : 2 * d, :], in_=q_sb[0:d, :])
            k_sb = persist.tile([P, hw], F16, name="k_sb")
            nc.sync.dma_start(out=k_sb[0:d, :], in_=qk_in[d : 2 * d, :])
            nc.sync.dma_start(out=k_sb[d : 2 * d, :], in_=k_sb[0:d, :])

            vt_sb = persist.tile([P, NPC, c], BF16, name="vt_sb")  # V^T

            # ---- phase 1: V load, upcast (channel-scaled), transpose ----
            with tc.tile_pool(name="vload", bufs=1) as vload:
                vsc_sb = vload.tile([P, NKC, 1], F32, name="vsc_sb")
                nc.sync.dma_start(
                    out=vsc_sb,
                    in_=vsc_in[:, :].rearrange("(a p) q -> p a q", p=P),
                )
                v8t = vload.tile([P, NKC, hw], I8, name="v8t")
                vr = v8_in[:, :].rearrange("(a p) q -> p a q", p=P)
                for kc in range(NKC):
                    nc.sync.dma_start(
                        out=v8t[:, kc : kc + 1, :], in_=vr[:, kc : kc + 1, :]
                    )
                vb = vload.tile([P, NKC, hw], BF16, name="vb")
                for kc in range(NKC):
                    nc.scalar.activation(
                        vb[:, kc, :],
                        v8t[:, kc, :],
                        AF.Copy,
                        scale=vsc_sb[:, kc, :],
                    )
                for pc in range(NPC):
                    tp = psT.tile([P, c], BF16, name="vt_ps", tag="psT")
                    for kc in range(NKC):
                        nc.tensor.transpose(
                            tp[:, kc * P : (kc + 1) * P],
                            vb[:, kc, pc * P : (pc + 1) * P],
                            ident,
                        )
                    nc.vector.tensor_copy(vt_sb[:, pc, :], tp)

            # ---- phase 2: attention (software-pipelined q-supers) ----
            with (
                tc.tile_pool(name="pp", bufs=2 * QT_PER_QS + 1) as pp,
                tc.tile_pool(name="ptp", bufs=NPC + 2) as ptp,
                tc.tile_pool(name="outp", bufs=4) as outp,
                tc.tile_pool(name="psS", bufs=2, space="PSUM") as psS,
            ):
                def produce(qs):
                    """S + exp + normalize for q-super qs; returns P tiles."""
                    p_tiles = []
                    for qt in range(QT_PER_QS):
                        qg = qs * QT_PER_QS + qt
                        qsl = slice(qg * P, (qg + 1) * P)
                        p_t = pp.tile([P, hw], BF16, name="p_t", tag="p")
                        l8 = small.tile([P, N_SH], F32, name="l8", tag="l8")
                        for sh in range(N_SH):
                            sp = psS.tile([P, S_W], F32, name="s_ps", tag="psS")
                            for j in range(S_W // 512):
                                pb = sh * (S_W // 512) + j
                                h = (pb % 2) * d
                                nc.tensor.matmul(
                                    sp[:, j * 512 : (j + 1) * 512],
                                    q_sb[h : h + d, qsl],
                                    k_sb[h : h + d, pb * 512 : (pb + 1) * 512],
                                    start=True,
                                    stop=True,
                                )
                            nc.scalar.activation(
                                p_t[:, sh * S_W : (sh + 1) * S_W],
                                sp,
                                AF.Exp,
                                accum_out=l8[:, sh : sh + 1],
                            )
                        lsum = small.tile([P, 1], F32, name="lsum", tag="lsum")
                        nc.vector.reduce_sum(lsum, l8, axis=AX.X)
                        rinv = small.tile([P, 1], F32, name="rinv", tag="rinv")
                        nc.vector.reciprocal(rinv, lsum)
                        nc.vector.tensor_scalar_mul(p_t, p_t, rinv)
                        p_tiles.append(p_t)
                    return p_tiles

                def consume(p_tiles, qs):
                    """P^T transposes + outT matmuls + int8 quantize + DMA."""
                    pt_tiles = []
                    for pc in range(NPC):
                        tp = psT.tile([P, 512], BF16, name="pt_ps", tag="psT")
                        for qt in range(QT_PER_QS):
                            nc.tensor.transpose(
                                tp[:, qt * P : (qt + 1) * P],
                                p_tiles[qt][:, pc * P : (pc + 1) * P],
                                ident,
                            )
                        pt_sb = ptp.tile([P, 512], BF16, name="pt_sb", tag="pt")
                        nc.vector.tensor_copy(pt_sb, tp)
                        pt_tiles.append(pt_sb)

                    for qt in range(QT_PER_QS):
                        qg = qs * QT_PER_QS + qt
                        ops = psV.tile([P, c], F32, name="pv_ps", tag="psV")
                        for pc in range(NPC):
                            nc.tensor.matmul(
                                ops,
                                pt_tiles[pc][:, qt * P : (qt + 1) * P],
                                vt_sb[:, pc, :],
                                start=(pc == 0),
                                stop=(pc == NPC - 1),
                            )
                        # per-query int8: rq = 127/absmax(row); o8 = rne(x*rq)
                        am = small.tile([P, 1], F32, name="am", tag="am")
                        nc.vector.tensor_reduce(
                            out=am,
                            in_=ops,
                            op=mybir.AluOpType.max,
                            axis=AX.X,
                            apply_absolute_value=True,
                        )
                        nc.vector.tensor_scalar_max(am, am, 1e-20)
                        rqv = outp.tile([P, 1], F32, name="rqv", tag="rqv")
                        nc.vector.reciprocal(rqv, am)
                        nc.vector.tensor_scalar_mul(rqv, rqv, 127.0)
                        o8t = outp.tile([P, c + 4], I8, name="o8t", tag="o8t")
                        nc.vector.tensor_scalar_mul(o8t[:, 0:c], ops, rqv)
                        nc.vector.tensor_copy(
                            o8t[:, c : c + 4], rqv.bitcast(I8)
                        )
                        nc.sync.dma_start(
                            out=o8_out[qg * P : (qg + 1) * P, :], in_=o8t
                        )

                prev = None
                for qs in range(NSLAB):
                    cur = produce(qs)
                    if prev is not None:
                        consume(*prev)
                    prev = (cur, qs)
                consume(*prev)

    nc.compile()
    return nc


# ---------------------------------------------------------------------------
# dispatch: trimmed run_bass_via_pjrt with cached jit + device-resident zeros
# ---------------------------------------------------------------------------

_STATE = {}


def _cpu():
    return jax.devices("cpu")[0]


def _get_state():
    if "groups" in _STATE:
        return _STATE

    install_neuronx_cc_hook()
    nc = build_nc()

    partition_name = (
        nc.partition_id_tensor.name if nc.partition_id_tensor else None
    )
    in_names = []
    out_names = []
    out_avals = []
    for alloc in nc.m.functions[0].allocations:
        if not isinstance(alloc, mybir.MemoryLocationSet):
            continue
        name = alloc.memorylocations[0].name
        if alloc.kind == "ExternalInput":
            if name != partition_name:
                in_names.append(name)
        elif alloc.kind == "ExternalOutput":
            out_names.append(name)
            out_avals.append(
                jax.core.ShapedArray(
                    tuple(alloc.tensor_shape), mybir.dt.np(alloc.dtype)
                )
            )
    all_in_names = in_names + out_names
    if partition_name is not None:
        all_in_names.append(partition_name)
    all_in_names = tuple(all_in_names)
    out_avals = tuple(out_avals)
    out_names = tuple(out_names)

    def _body(*args):
        operands = list(args)
        if partition_name is not None:
            operands.append(partition_id_tensor())
        outs = _bass_exec_p.bind(
            *operands,
            out_avals=out_avals,
            in_names=all_in_names,
            out_names=out_names,
            lowering_input_output_aliases=(),
            sim_require_finite=True,
            sim_require_nnan=True,
            nc=nc,
        )
        return tuple(outs)

    devices = jax.devices()[:N_CORES]
    n_args = len(in_names) + len(out_names)
    cpg = N_CORES // N_GROUPS
    groups = []
    for g in range(N_GROUPS):
        mesh = Mesh(np.asarray(devices[g * cpg : (g + 1) * cpg]), ("core",))
        sharded = jax.jit(
            shard_map(
                _body,
                mesh=mesh,
                in_specs=(PartitionSpec("core"),) * n_args,
                out_specs=(PartitionSpec("core"),) * len(out_names),
                check_rep=False,
            ),
            keep_unused=True,
        )
        # Dead "pre-zeroed output" operand the bass_exec convention
        # requires. Kept resident on device; never donated, so reusable.
        zshard = NamedSharding(mesh, PartitionSpec("core"))
        zo8 = jax.jit(
            lambda: jnp.zeros((cpg * HW, OC), jnp.int8), out_shardings=zshard
        )()
        groups.append((sharded, zo8))

    _STATE.update(
        groups=groups, cpg=cpg, in_names=in_names, nc=nc,
        pool=ThreadPoolExecutor(N_CORES),
    )
    return _STATE


def _prep_group(qf, kf, Wq, bq, Wk):
    # bk is dropped: softmax over keys is invariant to the per-query
    # constant q_i . bk that the K bias adds to every logit in a row.
    nb = qf.shape[0]
    Xq = qf.reshape(nb, C, HW)
    Xk = kf.reshape(nb, C, HW)
    Q = jnp.einsum("bcp,dc->bdp", Xq, Wq) + bq[None, :, None]
    K = jnp.einsum("bcp,dc->bdp", Xk, Wk)
    qk = jnp.concatenate([Q[:, None], K[:, None]], axis=1)  # (nb,2,D,HW)
    qkg = qk.astype(jnp.float16).reshape(nb * 2 * D, HW)
    amax = jnp.maximum(
        jnp.max(jnp.abs(Xk), axis=2, keepdims=True), 1e-20
    )  # (nb,C,1)
    v8 = (
        jnp.clip(jnp.round(Xk * (127.0 / amax)), -127, 127)
        .astype(jnp.int8)
        .reshape(nb * C, HW)
    )
    return qkg, v8, (amax / 127.0).reshape(nb * C, 1)


_PREP_GROUP = jax.jit(_prep_group)


def kernel(query_features, key_features, Wq, bq, Wk, bk, vis_CA=0, **_unused):
    t0 = time.time()
    st = _get_state()
    t1 = time.time()

    qf = np.asarray(query_features, np.float32)
    kf = np.asarray(key_features, np.float32)
    Wq_ = np.asarray(Wq, np.float32)
    bq_ = np.asarray(bq, np.float32)
    Wk_ = np.asarray(Wk, np.float32)
    t2 = time.time()

    # Per-group pipeline: prep group g on the CPU while group g-1's H2D
    # streams; D2H of early groups overlaps H2D of later ones (the tunnel
    # is full-duplex); shard fetch + host post pipeline under the streams.
    cpg = st["cpg"]
    futs = []
    with jax.default_device(_cpu()):
        for g, (sharded, zo8) in enumerate(st["groups"]):
            sl = slice(g * cpg, (g + 1) * cpg)
            qkg, v8g, vscg = _PREP_GROUP(qf[sl], kf[sl], Wq_, bq_, Wk_)
            (o8c_g,) = sharded(
                np.asarray(qkg), np.asarray(v8g), np.asarray(vscg), zo8
            )
            futs.extend(
                st["pool"].submit(lambda s: np.asarray(s.data), sh)
                for sh in o8c_g.addressable_shards
            )
    out = np.empty((B, C, HW), np.float32)
    t3 = None
    for b, fut in enumerate(futs):
        ob = fut.result()  # (HW, C+4) int8
        if t3 is None:
            t3 = time.time()
        rinv = 1.0 / ob[:, C : C + 4].copy().view(np.float32)  # (HW, 1)
        tmp = ob[:, :C].T.astype(np.float32)  # (C, HW)
        np.multiply(tmp, rinv.reshape(1, HW), out=out[b])
    t4 = time.time()

    _TIMINGS.update(
        setup=t1 - t0,
        prep=t2 - t1,
        device=(t3 or t4) - t2,
        fetch_post=t4 - (t3 or t4),
    )
    return out.reshape(B, C, 64, 64)


# revision 19
# speedup vs baseline: 2.0541x; 1.1351x over previous
"""Cross-attention layer for Trainium2 (Bass), 8-core data-parallel.

The wall-clock of a call is dominated by host<->device transfers over the
axon tunnel (~30-60 MB/s, partially full-duplex), not device compute
(~0.4 ms/core).  So the kernel is built around minimizing bytes and
round-trips on the wire:

  host (1 cpu, cheap): Q = Wq@Xq+bq, K = Wk@Xk+bk   (0.26% of FLOPs)
                       V -> per-channel int8 (scale amax_c/127)
  H2D per core:        qk fp16 [128,4096] (1 MB), v8 int8 (2 MB)
  device per core:     S = Q^T K (fp16 matmuls, f32 psum)
                       P = softmax(S) (exp w/ accumulated row sums, bf16)
                       outT[q,c] = sum_p P^T[p,q] V8^T[p,c] (bf16, f32 psum)
                       per-query int8 quantization of outT rows; the f32
                       quant multiplier rq is bitcast-packed into the last
                       4 columns so one D2H tensor carries everything
  D2H per core:        o8c int8 [4096, 516] (2.02 MB)
  host:                out[b] = (o8 / rq).T * vscale, pipelined per shard
                       under the D2H stream

Per-query (not per-channel) output scaling matters: attention rows vary
wildly in sharpness, so a channel-wide scale clips diffuse queries. The
device ships back its actual quantization multiplier rq (not a recomputed
reciprocal) so reciprocal-approximation error cancels exactly.

Dispatch is a trimmed run_bass_via_pjrt: one jit(shard_map) over 8 cores
cached at module level (no per-call retrace), with the dead "donated zero
output" operand kept resident on device so no zero bytes cross the tunnel.
"""

import time
from concurrent.futures import ThreadPoolExecutor

import numpy as np

try:
    import concourse.bass as bass  # noqa: F401
except ImportError:  # pragma: no cover - path setup for bare containers
    import sys

    sys.path.insert(0, "/opt/trn_rl_repo")
    import concourse.bass as bass  # noqa: F401

import jax
import jax.numpy as jnp
from jax.experimental.shard_map import shard_map
from jax.sharding import Mesh, NamedSharding, PartitionSpec

import concourse.mybir as mybir
import concourse.tile as tile
from concourse import bacc
from concourse.bass2jax import (
    _bass_exec_p,
    install_neuronx_cc_hook,
    partition_id_tensor,
)
from concourse.masks import make_identity

F32 = mybir.dt.float32
F16 = mybir.dt.float16
BF16 = mybir.dt.bfloat16
I8 = mybir.dt.int8
AF = mybir.ActivationFunctionType
AX = mybir.AxisListType

B = 8
C = 512
HW = 4096
D = 64
N_CORES = 8
OC = C + 4  # o8 columns + packed f32 rq
N_GROUPS = 8  # dispatch split: groups of cores, pipelined for duplex overlap

_TIMINGS = {}


def build_nc(c=C, hw=HW, d=D):
    """Single-core Bass program (SPMD across cores via shard_map)."""
    P = 128
    NKC = c // P          # 128-channel chunks of V
    NSLAB = hw // 512     # 512-wide q-supers
    NPC = hw // P         # 128-wide pixel chunks (transpose granularity)
    QT_PER_QS = 4         # 128-row q-tiles per q-super
    S_W = 1024            # S psum tile width
    N_SH = hw // S_W

    nc = bacc.Bacc("TRN2", target_bir_lowering=False)

    qk_in = nc.dram_tensor("qk", [2 * d, hw], F16, kind="ExternalInput")
    v8_in = nc.dram_tensor("v8", [c, hw], I8, kind="ExternalInput")
    vsc_in = nc.dram_tensor("vsc", [c, 1], F32, kind="ExternalInput")
    o8_out = nc.dram_tensor("o8c", [hw, c + 4], I8, kind="ExternalOutput")

    with tile.TileContext(nc) as tc:
        with (
            tc.tile_pool(name="const", bufs=1) as const,
            tc.tile_pool(name="persist", bufs=1) as persist,
            tc.tile_pool(name="small", bufs=4) as small,
            tc.tile_pool(name="psT", bufs=2, space="PSUM") as psT,
            tc.tile_pool(name="psV", bufs=2, space="PSUM") as psV,
        ):
            ident = const.tile([P, P], BF16, name="ident")
            make_identity(nc, ident)

            # Q/K in fp16, duplicated to both 64-row halves so S matmuls can
            # alternate PE array halves (overlaps weight load with streaming).
            q_sb = persist.tile([P, hw], F16, name="q_sb")
            nc.sync.dma_start(out=q_sb[0:d, :], in_=qk_in[0:d, :])
            nc.sync.dma_start(out=q_sb[d : 2 * d, :], in_=q_sb[0:d, :])
            k_sb = persist.tile([P, hw], F16, name="k_sb")
            nc.sync.dma_start(out=k_sb[0:d, :], in_=qk_in[d : 2 * d, :])
            nc.sync.dma_start(out=k_sb[d : 2 * d, :], in_=k_sb[0:d, :])

            vt_sb = persist.tile([P, NPC, c], BF16, name="vt_sb")  # V^T

            # ---- phase 1: V load, upcast (channel-scaled), transpose ----
            with tc.tile_pool(name="vload", bufs=1) as vload:
                vsc_sb = vload.tile([P, NKC, 1], F32, name="vsc_sb")
                nc.sync.dma_start(
                    out=vsc_sb,
                    in_=vsc_in[:, :].rearrange("(a p) q -> p a q", p=P),
                )
                v8t = vload.tile([P, NKC, hw], I8, name="v8t")
                vr = v8_in[:, :].rearrange("(a p) q -> p a q", p=P)
                for kc in range(NKC):
                    nc.sync.dma_start(
                        out=v8t[:, kc : kc + 1, :], in_=vr[:, kc : kc + 1, :]
                    )
                vb = vload.tile([P, NKC, hw], BF16, name="vb")
                for kc in range(NKC):
                    nc.scalar.activation(
                        vb[:, kc, :],
                        v8t[:, kc, :],
                        AF.Copy,
                        scale=vsc_sb[:, kc, :],
                    )
                for pc in range(NPC):
                    tp = psT.tile([P, c], BF16, name="vt_ps", tag="psT")
                    for kc in range(NKC):
                        nc.tensor.transpose(
                            tp[:, kc * P : (kc + 1) * P],
                            vb[:, kc, pc * P : (pc + 1) * P],
                            ident,
                        )
                    nc.vector.tensor_copy(vt_sb[:, pc, :], tp)

            # ---- phase 2: attention (software-pipelined q-supers) ----
            with (
                tc.tile_pool(name="pp", bufs=2 * QT_PER_QS + 1) as pp,
                tc.tile_pool(name="ptp", bufs=NPC + 2) as ptp,
                tc.tile_pool(name="outp", bufs=4) as outp,
                tc.tile_pool(name="psS", bufs=2, space="PSUM") as psS,
            ):
                def produce(qs):
                    """S + exp + normalize for q-super qs; returns P tiles."""
                    p_tiles = []
                    for qt in range(QT_PER_QS):
                        qg = qs * QT_PER_QS + qt
                        qsl = slice(qg * P, (qg + 1) * P)
                        p_t = pp.tile([P, hw], BF16, name="p_t", tag="p")
                        l8 = small.tile([P, N_SH], F32, name="l8", tag="l8")
                        for sh in range(N_SH):
                            sp = psS.tile([P, S_W], F32, name="s_ps", tag="psS")
                            for j in range(S_W // 512):
                                pb = sh * (S_W // 512) + j
                                h = (pb % 2) * d
                                nc.tensor.matmul(
                                    sp[:, j * 512 : (j + 1) * 512],
                                    q_sb[h : h + d, qsl],
                                    k_sb[h : h + d, pb * 512 : (pb + 1) * 512],
                                    start=True,
                                    stop=True,
                                )
                            nc.scalar.activation(
                                p_t[:, sh * S_W : (sh + 1) * S_W],
                                sp,
                                AF.Exp,
                                accum_out=l8[:, sh : sh + 1],
                            )
                        lsum = small.tile([P, 1], F32, name="lsum", tag="lsum")
                        nc.vector.reduce_sum(lsum, l8, axis=AX.X)
                        rinv = small.tile([P, 1], F32, name="rinv", tag="rinv")
                        nc.vector.reciprocal(rinv, lsum)
                        nc.vector.tensor_scalar_mul(p_t, p_t, rinv)
                        p_tiles.append(p_t)
                    return p_tiles

                def consume(p_tiles, qs):
                    """P^T transposes + outT matmuls + int8 quantize + DMA."""
                    pt_tiles = []
                    for pc in range(NPC):
                        tp = psT.tile([P, 512], BF16, name="pt_ps", tag="psT")
                        for qt in range(QT_PER_QS):
                            nc.tensor.transpose(
                                tp[:, qt * P : (qt + 1) * P],
                                p_tiles[qt][:, pc * P : (pc + 1) * P],
                                ident,
                            )
                        pt_sb = ptp.tile([P, 512], BF16, name="pt_sb", tag="pt")
                        nc.vector.tensor_copy(pt_sb, tp)
                        pt_tiles.append(pt_sb)

                    for qt in range(QT_PER_QS):
                        qg = qs * QT_PER_QS + qt
                        ops = psV.tile([P, c], F32, name="pv_ps", tag="psV")
                        for pc in range(NPC):
                            nc.tensor.matmul(
                                ops,
                                pt_tiles[pc][:, qt * P : (qt + 1) * P],
                                vt_sb[:, pc, :],
                                start=(pc == 0),
                                stop=(pc == NPC - 1),
                            )
                        # per-query int8: rq = 127/absmax(row); o8 = rne(x*rq)
                        am = small.tile([P, 1], F32, name="am", tag="am")
                        nc.vector.tensor_reduce(
                            out=am,
                            in_=ops,
                            op=mybir.AluOpType.max,
                            axis=AX.X,
                            apply_absolute_value=True,
                        )
                        nc.vector.tensor_scalar_max(am, am, 1e-20)
                        rqv = outp.tile([P, 1], F32, name="rqv", tag="rqv")
                        nc.vector.reciprocal(rqv, am)
                        nc.vector.tensor_scalar_mul(rqv, rqv, 127.0)
                        o8t = outp.tile([P, c + 4], I8, name="o8t", tag="o8t")
                        nc.vector.tensor_scalar_mul(o8t[:, 0:c], ops, rqv)
                        nc.vector.tensor_copy(
                            o8t[:, c : c + 4], rqv.bitcast(I8)
                        )
                        nc.sync.dma_start(
                            out=o8_out[qg * P : (qg + 1) * P, :], in_=o8t
                        )

                prev = None
                for qs in range(NSLAB):
                    cur = produce(qs)
                    if prev is not None:
                        consume(*prev)
                    prev = (cur, qs)
                consume(*prev)

    nc.compile()
    return nc


# ---------------------------------------------------------------------------
# dispatch: trimmed run_bass_via_pjrt with cached jit + device-resident zeros
# ---------------------------------------------------------------------------

_STATE = {}


def _cpu():
    return jax.devices("cpu")[0]


def _get_state():
    if "groups" in _STATE:
        return _STATE

    install_neuronx_cc_hook()
    nc = build_nc()

    partition_name = (
        nc.partition_id_tensor.name if nc.partition_id_tensor else None
    )
    in_names = []
    out_names = []
    out_avals = []
    for alloc in nc.m.functions[0].allocations:
        if not isinstance(alloc, mybir.MemoryLocationSet):
            continue
        name = alloc.memorylocations[0].name
        if alloc.kind == "ExternalInput":
            if name != partition_name:
                in_names.append(name)
        elif alloc.kind == "ExternalOutput":
            out_names.append(name)
            out_avals.append(
                jax.core.ShapedArray(
                    tuple(alloc.tensor_shape), mybir.dt.np(alloc.dtype)
                )
            )
    all_in_names = in_names + out_names
    if partition_name is not None:
        all_in_names.append(partition_name)
    all_in_names = tuple(all_in_names)
    out_avals = tuple(out_avals)
    out_names = tuple(out_names)

    def _body(*args):
        operands = list(args)
        if partition_name is not None:
            operands.append(partition_id_tensor())
        outs = _bass_exec_p.bind(
            *operands,
            out_avals=out_avals,
            in_names=all_in_names,
            out_names=out_names,
            lowering_input_output_aliases=(),
            sim_require_finite=True,
            sim_require_nnan=True,
            nc=nc,
        )
        return tuple(outs)

    devices = jax.devices()[:N_CORES]
    n_args = len(in_names) + len(out_names)
    cpg = N_CORES // N_GROUPS
    groups = []
    for g in range(N_GROUPS):
        mesh = Mesh(np.asarray(devices[g * cpg : (g + 1) * cpg]), ("core",))
        sharded = jax.jit(
            shard_map(
                _body,
                mesh=mesh,
                in_specs=(PartitionSpec("core"),) * n_args,
                out_specs=(PartitionSpec("core"),) * len(out_names),
                check_rep=False,
            ),
            keep_unused=True,
        )
        # Dead "pre-zeroed output" operand the bass_exec convention
        # requires. Kept resident on device; never donated, so reusable.
        zshard = NamedSharding(mesh, PartitionSpec("core"))
        zo8 = jax.jit(
            lambda: jnp.zeros((cpg * HW, OC), jnp.int8), out_shardings=zshard
        )()
        groups.append((sharded, zo8))

    _STATE.update(
        groups=groups, cpg=cpg, in_names=in_names, nc=nc,
        pool=ThreadPoolExecutor(N_CORES),
    )
    return _STATE


def _prep_group(qf, kf, Wq, bq, Wk):
    # bk is dropped: softmax over keys is invariant to the per-query
    # constant q_i . bk that the K bias adds to every logit in a row.
    nb = qf.shape[0]
    Xq = qf.reshape(nb, C, HW)
    Xk = kf.reshape(nb, C, HW)
    Q = jnp.einsum("bcp,dc->bdp", Xq, Wq) + bq[None, :, None]
    K = jnp.einsum("bcp,dc->bdp", Xk, Wk)
    qk = jnp.concatenate([Q[:, None], K[:, None]], axis=1)  # (nb,2,D,HW)
    qkg = qk.astype(jnp.float16).reshape(nb * 2 * D, HW)
    amax = jnp.maximum(
        jnp.max(jnp.abs(Xk), axis=2, keepdims=True), 1e-20
    )  # (nb,C,1)
    v8 = (
        jnp.clip(jnp.round(Xk * (127.0 / amax)), -127, 127)
        .astype(jnp.int8)
        .reshape(nb * C, HW)
    )
    return qkg, v8, (amax / 127.0).reshape(nb * C, 1)


_PREP_GROUP = jax.jit(_prep_group)


def kernel(query_features, key_features, Wq, bq, Wk, bk, vis_CA=0, **_unused):
    t0 = time.time()
    st = _get_state()
    t1 = time.time()

    qf = np.asarray(query_features, np.float32)
    kf = np.asarray(key_features, np.float32)
    Wq_ = np.asarray(Wq, np.float32)
    bq_ = np.asarray(bq, np.float32)
    Wk_ = np.asarray(Wk, np.float32)
    t2 = time.time()

    # Per-group pipeline: prep group g on the CPU while group g-1's H2D
    # streams; D2H of early groups overlaps H2D of later ones (the tunnel
    # is full-duplex); shard fetch + host post pipeline under the streams.
    cpg = st["cpg"]
    futs = []
    with jax.default_device(_cpu()):
        for g, (sharded, zo8) in enumerate(st["groups"]):
            sl = slice(g * cpg, (g + 1) * cpg)
            qkg, v8g, vscg = _PREP_GROUP(qf[sl], kf[sl], Wq_, bq_, Wk_)
            (o8c_g,) = sharded(
                np.asarray(qkg), np.asarray(v8g), np.asarray(vscg), zo8
            )
            futs.extend(
                st["pool"].submit(lambda s: np.asarray(s.data), sh)
                for sh in o8c_g.addressable_shards
            )
    out = np.empty((B, C, HW), np.float32)
    t3 = None
    for b, fut in enumerate(futs):
        ob = fut.result()  # (HW, C+4) int8
        if t3 is None:
            t3 = time.time()
        rinv = 1.0 / ob[:, C : C + 4].copy().view(np.float32)  # (HW, 1)
        tmp = ob[:, :C].T.astype(np.float32)  # (C, HW)
        np.multiply(tmp, rinv.reshape(1, HW), out=out[b])
    t4 = time.time()

    _TIMINGS.update(
        setup=t1 - t0,
        prep=t2 - t1,
        device=(t3 or t4) - t2,
        fetch_post=t4 - (t3 or t4),
    )
    return out.reshape(B, C, 64, 64)
